# revision 1
# baseline (speedup 1.0000x reference)
"""Trainium2 Bass kernel for the arc-projection problem.

Full-input contract: kernel(**inputs) takes the unsharded numpy inputs and
returns the full output. Internally shards the batch N=64 across 8 cores
(pure data parallel), runs one SPMD Bass kernel, and gathers.

Algorithm (matches reference._arc_projection exactly, reformulated to be
gather-free):
  For each (sample, branch, direction) row:
    - segment vectors sv, masked lengths sl, exclusive cumsum cum (scan op)
    - project trajectory point 0 on all segments -> entry_s (one-hot argmin)
    - target_s[t] = clip(entry_s + traj_cum[t], 0, total)
    - proj(s) = rp[first_valid] + sum_j (sv_j/sl_j) * clip(s - cum_j, 0, sl_j)
      which equals the reference's searchsorted+lerp for prefix/suffix-true
      masks (invalid segments contribute exactly 0).
    - cost = sum_t |pos_t - proj_t|; per-sample argmin over 32 rows via
      one-hot and a masked cross-partition reduction in t-major layout.
"""

import sys

import numpy as np

try:
    import concourse.bass as bass
except ImportError:  # pragma: no cover - container without PYTHONPATH set
    sys.path.insert(0, "/opt/trn_rl_repo")
    import concourse.bass as bass

import concourse.tile as tile
from concourse import bacc, mybir
from concourse.bass_utils import run_bass_kernel_spmd

f32 = mybir.dt.float32
AT = mybir.AluOpType
AX = mybir.AxisListType

N, T, NB, NP = 64, 128, 16, 256
NCORES = 8
NS = N // NCORES          # samples per core
NB2 = 2 * NB              # fwd + bwd branches
NSEG = NP - 1
BIG = 1.0e30
RT = 128                  # rows per partition-tile


def _view(t, ap_dims, extra_off=0):
    """Strided view of a tile/AP: ap_dims are [step, count] free dims after
    the partition dim (kept from t)."""
    return bass.AP(tensor=t.tensor, offset=t.offset + extra_off,
                   ap=[t.ap[0]] + ap_dims)


def _dview(t, ap_dims, extra_off=0):
    """Raw view of a DRAM tile: ap_dims replace all dims."""
    return bass.AP(tensor=t.tensor, offset=t.offset + extra_off, ap=ap_dims)


def build_nc(ns=NS, enable_asserts=False, gp_t=0):
    """Build the per-core Bass program for ns samples. gp_t trajectory steps
    per tile are computed on GPSIMD instead of DVE (engine parallelism)."""
    rows = ns * NB2
    ntiles = (rows + RT - 1) // RT
    spt = RT // NB2  # samples per tile

    nc = bacc.Bacc("TRN2", target_bir_lowering=False, debug=False,
                   enable_asserts=enable_asserts, num_devices=NCORES)

    rp_d = nc.dram_tensor("rp", [rows, 3, NP], f32, kind="ExternalInput")
    mk_d = nc.dram_tensor("mk", [rows, NP], f32, kind="ExternalInput")
    tj_d = nc.dram_tensor("tj", [ns, 3, T], f32, kind="ExternalInput")
    out_d = nc.dram_tensor("out", [ns, T, 3], f32, kind="ExternalOutput")

    with tile.TileContext(nc) as tc:
        with (
            tc.tile_pool(name="work", bufs=2) as wp,
            tc.tile_pool(name="fin", bufs=2) as fp,
            tc.tile_pool(name="dram", bufs=1, space="DRAM") as dp,
        ):
            proj_s = dp.tile([rows, 3, T], f32)
            cost_s = dp.tile([rows], f32)
            oh_s = dp.tile([rows], f32)

            for k in range(ntiles):
                p = min(RT, rows - k * RT)
                r0 = k * RT

                rpt = wp.tile([p, 3, NP], f32, tag="rpt")
                nc.sync.dma_start(out=rpt, in_=rp_d.ap()[r0:r0 + p])
                mt = wp.tile([p, NP], f32, tag="mt")
                nc.sync.dma_start(out=mt, in_=mk_d.ap()[r0:r0 + p])
                # trajectory of each row's sample, broadcast to its 32 rows
                tpb = wp.tile([p, 3, T], f32, tag="tpb")
                nc.sync.dma_start(out=tpb, in_=bass.AP(
                    tensor=tj_d.ap().tensor, offset=k * spt * 3 * T,
                    ap=[[3 * T, spt], [0, NB2], [1, 3 * T]]))

                # --- segment data ---
                sv = wp.tile([p, 3, NSEG], f32, tag="sv")
                nc.vector.tensor_sub(out=sv, in0=rpt[:, :, 1:NP],
                                     in1=rpt[:, :, 0:NSEG])
                sm = wp.tile([p, NSEG], f32, tag="sm")
                nc.vector.tensor_mul(out=sm, in0=mt[:, 1:NP], in1=mt[:, 0:NSEG])
                sq3 = wp.tile([p, 3, NSEG], f32, tag="sq3")
                nc.vector.tensor_mul(out=sq3, in0=sv, in1=sv)
                sl2 = wp.tile([p, NSEG], f32, tag="sl2")
                nc.vector.tensor_reduce(out=sl2,
                                        in_=_view(sq3, [[1, NSEG], [NSEG, 3]]),
                                        axis=AX.X, op=AT.add)
                sl2m = wp.tile([p, NSEG], f32, tag="sl2m")
                nc.vector.tensor_mul(out=sl2m, in0=sl2, in1=sm)
                sl = wp.tile([p, NSEG], f32, tag="sl")
                nc.scalar.sqrt(out=sl, in_=sl2m)

                cum = wp.tile([p, NP], f32, tag="cum")
                zc = wp.tile([p, 1], f32, tag="zc")
                nc.vector.memset(zc, 0.0)
                nc.vector.memset(cum[:, 0:1], 0.0)
                nc.vector.tensor_tensor_scan(
                    out=cum[:, 1:NP], data0=sl, data1=_view(zc, [[0, NSEG]]),
                    initial=0.0, op0=AT.add, op1=AT.add)
                total = cum[:, NP - 1:NP]
                cumneg = wp.tile([p, NSEG], f32, tag="cumneg")
                nc.vector.tensor_scalar(out=cumneg, in0=cum[:, 0:NSEG],
                                        scalar1=-1.0, scalar2=None, op0=AT.mult)
                slmax = wp.tile([p, NSEG], f32, tag="slmax")
                nc.vector.tensor_scalar(out=slmax, in0=sl, scalar1=1e-9,
                                        scalar2=None, op0=AT.max)
                rsl = wp.tile([p, NSEG], f32, tag="rsl")
                nc.vector.reciprocal(out=rsl, in_=slmax)
                w = wp.tile([p, 3, NSEG], f32, tag="w")
                nc.vector.tensor_mul(out=w, in0=sv,
                                     in1=_view(rsl, [[0, 3], [1, NSEG]]))

                # --- project p0 on all segments; entry_s via one-hot argmin ---
                tmp3 = wp.tile([p, 3, NSEG], f32, tag="tmp3")
                for c in range(3):
                    # (a_c - p0_c) * sv_c
                    nc.vector.scalar_tensor_tensor(
                        out=tmp3[:, c, :], in0=rpt[:, c, 0:NSEG],
                        scalar=tpb[:, c, 0:1], in1=sv[:, c, :],
                        op0=AT.subtract, op1=AT.mult)
                dotn = wp.tile([p, NSEG], f32, tag="dotn")
                nc.vector.tensor_reduce(out=dotn,
                                        in_=_view(tmp3, [[1, NSEG], [NSEG, 3]]),
                                        axis=AX.X, op=AT.add)
                svd = wp.tile([p, NSEG], f32, tag="svd")
                nc.vector.tensor_scalar(out=svd, in0=sl2, scalar1=1e-12,
                                        scalar2=None, op0=AT.max)
                rsvd = wp.tile([p, NSEG], f32, tag="rsvd")
                nc.vector.reciprocal(out=rsvd, in_=svd)
                t0 = wp.tile([p, NSEG], f32, tag="t0")
                nc.vector.tensor_mul(out=t0, in0=dotn, in1=rsvd)
                # t0 = min(max(-t0, 0), 1)
                nc.vector.tensor_scalar(out=t0, in0=t0, scalar1=-1.0,
                                        scalar2=0.0, op0=AT.mult, op1=AT.max)
                nc.vector.tensor_scalar(out=t0, in0=t0, scalar1=1.0,
                                        scalar2=None, op0=AT.min)
                s3 = wp.tile([p, 3, NSEG], f32, tag="s3")
                nc.vector.tensor_mul(out=s3, in0=sv,
                                     in1=_view(t0, [[0, 3], [1, NSEG]]))
                e3 = wp.tile([p, 3, NSEG], f32, tag="e3")
                for c in range(3):
                    # (a_c - p0_c) + t0*sv_c  (= q0_c - p0_c)
                    nc.vector.scalar_tensor_tensor(
                        out=e3[:, c, :], in0=rpt[:, c, 0:NSEG],
                        scalar=tpb[:, c, 0:1], in1=s3[:, c, :],
                        op0=AT.subtract, op1=AT.add)
                e3sq = wp.tile([p, 3, NSEG], f32, tag="e3sq")
                nc.vector.tensor_mul(out=e3sq, in0=e3, in1=e3)
                d2 = wp.tile([p, NSEG], f32, tag="d2")
                nc.vector.tensor_reduce(out=d2,
                                        in_=_view(e3sq, [[1, NSEG], [NSEG, 3]]),
                                        axis=AX.X, op=AT.add)
                d2m = wp.tile([p, NSEG], f32, tag="d2m")
                # d2m = d2 + (1-sm)*BIG  (sm is exactly 0/1)
                nc.vector.tensor_scalar(out=d2m, in0=sm, scalar1=1.0,
                                        scalar2=-BIG, op0=AT.subtract,
                                        op1=AT.mult)
                nc.vector.tensor_add(out=d2m, in0=d2m, in1=d2)
                dmin = wp.tile([p, 1], f32, tag="dmin")
                nc.vector.tensor_reduce(out=dmin, in_=d2m, axis=AX.X, op=AT.min)
                ohseg = wp.tile([p, NSEG], f32, tag="ohseg")
                nc.vector.tensor_scalar(out=ohseg, in0=d2m, scalar1=dmin,
                                        scalar2=None, op0=AT.is_equal)
                # argmin ties are structural (projection clamped to a shared
                # vertex gives exactly-equal d2 on adjacent segments); keep
                # only the FIRST hot via prefix-max diff, matching jnp.argmin.
                pmax = wp.tile([p, NSEG], f32, tag="pmax")
                nc.vector.tensor_tensor_scan(
                    out=pmax, data0=ohseg, data1=_view(zc, [[0, NSEG]]),
                    initial=0.0, op0=AT.max, op1=AT.add)
                nc.vector.tensor_copy(out=ohseg[:, 0:1], in_=pmax[:, 0:1])
                nc.vector.tensor_sub(out=ohseg[:, 1:NSEG], in0=pmax[:, 1:NSEG],
                                     in1=pmax[:, 0:NSEG - 1])
                es = wp.tile([p, NSEG], f32, tag="es")
                nc.vector.tensor_mul(out=es, in0=t0, in1=sl)
                nc.vector.tensor_add(out=es, in0=es, in1=cum[:, 0:NSEG])
                entry = wp.tile([p, 1], f32, tag="entry")
                junk0 = wp.tile([p, NSEG], f32, tag="junk0")
                nc.vector.scalar_tensor_tensor(
                    out=junk0, in0=ohseg, scalar=1.0, in1=es,
                    op0=AT.mult, op1=AT.mult, accum_out=entry)

                # --- base point rp[first valid segment] ---
                ohf = wp.tile([p, NSEG], f32, tag="ohf")
                nc.vector.tensor_copy(out=ohf[:, 0:1], in_=sm[:, 0:1])
                nc.vector.tensor_sub(out=ohf[:, 1:NSEG], in0=sm[:, 1:NSEG],
                                     in1=sm[:, 0:NSEG - 1])
                nc.vector.tensor_scalar(out=ohf, in0=ohf, scalar1=0.0,
                                        scalar2=None, op0=AT.max)
                base3 = wp.tile([p, 3], f32, tag="base3")
                for c in range(3):
                    nc.vector.scalar_tensor_tensor(
                        out=junk0, in0=ohf, scalar=1.0, in1=rpt[:, c, 0:NSEG],
                        op0=AT.mult, op1=AT.mult,
                        accum_out=base3[:, c:c + 1])

                # --- trajectory cumulative arc length + target_s ---
                td = wp.tile([p, 3, T - 1], f32, tag="td")
                nc.vector.tensor_sub(out=td, in0=tpb[:, :, 1:T],
                                     in1=tpb[:, :, 0:T - 1])
                td2 = wp.tile([p, 3, T - 1], f32, tag="td2")
                nc.vector.tensor_mul(out=td2, in0=td, in1=td)
                tl2 = wp.tile([p, T - 1], f32, tag="tl2")
                nc.vector.tensor_reduce(out=tl2,
                                        in_=_view(td2, [[1, T - 1], [T - 1, 3]]),
                                        axis=AX.X, op=AT.add)
                tl = wp.tile([p, T - 1], f32, tag="tl")
                nc.scalar.sqrt(out=tl, in_=tl2)
                tcum = wp.tile([p, T], f32, tag="tcum")
                nc.vector.memset(tcum[:, 0:1], 0.0)
                nc.vector.tensor_tensor_scan(
                    out=tcum[:, 1:T], data0=tl, data1=_view(zc, [[0, T - 1]]),
                    initial=0.0, op0=AT.add, op1=AT.add)
                target = wp.tile([p, T], f32, tag="target")
                nc.vector.scalar_tensor_tensor(
                    out=target, in0=tcum, scalar=entry,
                    in1=_view(total, [[0, T]]), op0=AT.add, op1=AT.min)
                nc.vector.tensor_scalar(out=target, in0=target, scalar1=0.0,
                                        scalar2=None, op0=AT.max)

                # --- main pass: proj[c, t] = sum_j w_c*clip(s_t - cum_j, 0, sl_j)
                # t-range split between DVE and GPSIMD; each range gets its
                # own proj/cost, stored to the shared DRAM scratch slices.
                ranges = [(0, T - gp_t, nc.vector, "d")]
                if gp_t > 0:
                    ranges.append((T - gp_t, T, nc.gpsimd, "g"))
                cost_parts = []
                for (tlo, thi, eng, sx) in ranges:
                    tn = thi - tlo
                    proj = wp.tile([p, 3, tn], f32, tag="proj" + sx)
                    vt = wp.tile([p, NSEG], f32, tag="vt" + sx)
                    jx = wp.tile([p, NSEG], f32, tag="jx" + sx)
                    jy = wp.tile([p, NSEG], f32, tag="jy" + sx)
                    jz = wp.tile([p, NSEG], f32, tag="jz" + sx)
                    jt = (jx, jy, jz)
                    if sx == "g":
                        vm = wp.tile([p, NSEG], f32, tag="vm" + sx)
                    for t in range(tlo, thi):
                        i = t - tlo
                        if sx == "d":
                            # DVE: fused stt ops with accumulator reads
                            eng.scalar_tensor_tensor(
                                out=vt, in0=cumneg, scalar=target[:, t:t + 1],
                                in1=sl, op0=AT.add, op1=AT.min)
                            for c in range(3):
                                eng.scalar_tensor_tensor(
                                    out=jt[c], in0=vt, scalar=0.0,
                                    in1=w[:, c, :], op0=AT.max, op1=AT.mult,
                                    accum_out=proj[:, c, i:i + 1])
                        else:
                            # GPSIMD computes the hinge products (no stt on
                            # pool engine); ACT does the j-sum via Copy+accum.
                            eng.tensor_scalar(
                                out=vt, in0=cumneg,
                                scalar1=target[:, t:t + 1], scalar2=0.0,
                                op0=AT.add, op1=AT.max)
                            eng.tensor_tensor(out=vm, in0=vt, in1=sl,
                                              op=AT.min)
                            for c in range(3):
                                # rotating buffer so ACT reads overlap the
                                # next t's GPSIMD multiplies
                                mc = wp.tile([p, NSEG], f32, tag=f"m{c}g")
                                eng.tensor_mul(out=mc, in0=vm,
                                               in1=w[:, c, :])
                                nc.scalar.activation(
                                    out=jt[c], in_=mc,
                                    func=mybir.ActivationFunctionType.Copy,
                                    accum_out=proj[:, c, i:i + 1])

                    # add base, store proj slice, partial cost
                    for c in range(3):
                        nc.vector.tensor_scalar(
                            out=proj[:, c, :], in0=proj[:, c, :],
                            scalar1=base3[:, c:c + 1], scalar2=None, op0=AT.add)
                    nc.sync.dma_start(out=proj_s[r0:r0 + p, :, tlo:thi],
                                      in_=proj)
                    df = wp.tile([p, 3, tn], f32, tag="df" + sx)
                    nc.vector.tensor_sub(out=df, in0=proj,
                                         in1=tpb[:, :, tlo:thi])
                    df2 = wp.tile([p, 3, tn], f32, tag="df2" + sx)
                    nc.vector.tensor_mul(out=df2, in0=df, in1=df)
                    dd = wp.tile([p, tn], f32, tag="dd" + sx)
                    nc.vector.tensor_reduce(out=dd,
                                            in_=_view(df2, [[1, tn], [tn, 3]]),
                                            axis=AX.X, op=AT.add)
                    dist = wp.tile([p, tn], f32, tag="dist" + sx)
                    costp = wp.tile([p, 1], f32, tag="cost" + sx)
                    nc.scalar.activation(out=dist, in_=dd,
                                         func=mybir.ActivationFunctionType.Sqrt,
                                         accum_out=costp)
                    cost_parts.append(costp)
                cost = cost_parts[0]
                if len(cost_parts) > 1:
                    nc.vector.tensor_add(out=cost, in0=cost, in1=cost_parts[1])
                nc.sync.dma_start(
                    out=_dview(cost_s, [[1, p]], extra_off=r0), in_=cost)

            # --- per-sample argmin over branches ---
            costT = fp.tile([ns, NB2], f32)
            nc.sync.dma_start(out=costT,
                              in_=_dview(cost_s, [[NB2, ns], [1, NB2]]))
            cmin = fp.tile([ns, 1], f32)
            nc.vector.tensor_reduce(out=cmin, in_=costT, axis=AX.X, op=AT.min)
            oh8 = fp.tile([ns, NB2], f32)
            nc.vector.tensor_scalar(out=oh8, in0=costT, scalar1=cmin,
                                    scalar2=None, op0=AT.is_equal)
            zc8 = fp.tile([ns, 1], f32)
            nc.vector.memset(zc8, 0.0)
            pm8 = fp.tile([ns, NB2], f32)
            nc.vector.tensor_tensor_scan(
                out=pm8, data0=oh8, data1=_view(zc8, [[0, NB2]]),
                initial=0.0, op0=AT.max, op1=AT.add)
            nc.vector.tensor_copy(out=oh8[:, 0:1], in_=pm8[:, 0:1])
            nc.vector.tensor_sub(out=oh8[:, 1:NB2], in0=pm8[:, 1:NB2],
                                 in1=pm8[:, 0:NB2 - 1])
            nc.sync.dma_start(out=_dview(oh_s, [[NB2, ns], [1, NB2]]), in_=oh8)

            # --- gather best branch per sample in t-major layout ---
            for n in range(ns):
                pb = fp.tile([T, NB2, 3], f32, tag="pb")
                nc.sync.dma_start(out=pb, in_=bass.AP(
                    tensor=proj_s.tensor,
                    offset=proj_s.offset + n * NB2 * 3 * T,
                    ap=[[1, T], [3 * T, NB2], [T, 3]]))
                ohb = fp.tile([T, NB2], f32, tag="ohb")
                nc.sync.dma_start(out=ohb, in_=bass.AP(
                    tensor=oh_s.tensor, offset=oh_s.offset + n * NB2,
                    ap=[[0, T], [1, NB2]]))
                pm = fp.tile([T, NB2, 3], f32, tag="pm")
                nc.vector.tensor_mul(out=pm, in0=pb,
                                     in1=_view(ohb, [[1, NB2], [0, 3]]))
                outn = fp.tile([T, 3], f32, tag="outn")
                nc.vector.tensor_reduce(out=outn,
                                        in_=_view(pm, [[1, 3], [3, NB2]]),
                                        axis=AX.X, op=AT.add)
                nc.sync.dma_start(out=out_d.ap()[n], in_=outn)

    nc.compile()
    return nc


def marshal_inputs(selected_traj, road_points, road_mask):
    """Host-side layout marshaling (permutations/casts only): per-core input
    dicts with fwd+bwd branch rows and planar (xyz-major) layouts."""
    st = np.ascontiguousarray(selected_traj, dtype=np.float32)
    rp = np.ascontiguousarray(road_points, dtype=np.float32)
    rm = np.asarray(road_mask)

    rp_ext = np.concatenate([rp, rp[:, :, ::-1, :]], axis=1)        # [N,NB2,NP,3]
    rp_ext = np.ascontiguousarray(rp_ext.transpose(0, 1, 3, 2))     # [N,NB2,3,NP]
    mk_ext = np.concatenate([rm, rm[:, :, ::-1]], axis=1).astype(np.float32)
    tj = np.ascontiguousarray(st.transpose(0, 2, 1))                # [N,3,T]

    in_maps = []
    for c in range(NCORES):
        s = slice(c * NS, (c + 1) * NS)
        in_maps.append({
            "rp": np.ascontiguousarray(rp_ext[s]).reshape(NS * NB2, 3, NP),
            "mk": np.ascontiguousarray(mk_ext[s]).reshape(NS * NB2, NP),
            "tj": np.ascontiguousarray(tj[s]),
        })
    return in_maps


_NC = None


def kernel(selected_traj, road_points, road_mask):
    global _NC
    if _NC is None:
        _NC = build_nc()
    in_maps = marshal_inputs(selected_traj, road_points, road_mask)
    res = run_bass_kernel_spmd(_NC, in_maps, core_ids=list(range(NCORES)))
    out = np.concatenate([r["out"] for r in res.results], axis=0)
    return out.astype(np.float32)

